# revision 3
# baseline (speedup 1.0000x reference)
"""Multi-head attention (B=2, S=2048, D=1024, H=16, Hd=64) on 8 Trainium2
NeuronCores.

Sharding: 8 cores = (batch 2) x (head-quarter 4).  Core (b, hq) computes,
for batch b and heads hq*4..hq*4+3, the full-sequence partial output

    outp = (softmax-attention of its 4 heads over all 2048 q rows) @ Wo_part.T

and the host sums the four head-quarter partials per batch and adds bo.

Everything is bf16 on the wire and in SBUF (PSUM accumulates fp32).
Host-side layouts are all p-major so every DMA descriptor is >= 4KB:
  xC     [NJ, 128, KT, 512]   x[b].T j-chunked, 8KB runs
  w{q,k,v}P [128, KT, DPC]    W.T slice, 4KB runs
  woP    [128, 2, D]          Wo.T slice, 4KB runs
  maskP  [128, NSK, S]        keep-mask (mask[b,0]==0).T, 4KB runs/tile
  outp   [S, D]               bf16 partial, 2KB rows

DMA queues: the 8MB mask streams on the qAct HWDGE queue (issued while
ACT is still idle) in parallel with x/weights/outputs on qSP.

Execution is one software-pipelined slot stream over all 8 (c, j) units
(c = head pair, j = 512-wide q chunk), 16 s_k tiles each — no
per-unit pipeline drain.  Slot g: scores matmul pair -> exp (ScalarE,
the pacing engine) -> keep-mask multiply (VectorE, 2x bf16 mode);
attnV accumulates L slots behind.  Projection chains, softmax-
normalization multiplies, and phase-3 output blocks are emitted from a
deadline-ordered (EDF) work heap, at most a few sub-microsecond pieces
per slot, so the PE stays dense without stalling the exp stream.
Emission order IS dependency order for the tile framework, so every
piece carries an exclusive deadline (= first slot that consumes it)
checked at build time.

Head packing: a pair's two heads live on partition halves 0-63/64-127
so the pair's two scores matmuls run concurrently on disjoint PE row
groups.  V lands in v_aug [128, NSK, head*128 + (64 V | 64 ones)]; the
ones columns make the attnV matmul accumulate Z = sum(expm) into PSUM
rows 64..127 at no extra stream cost (the stream time is set by the
512 moving columns, not the stationary width).
"""

import sys

if "/opt/trn_rl_repo" not in sys.path:
    sys.path.insert(0, "/opt/trn_rl_repo")

import heapq

import numpy as np

B, S, D = 2, 2048, 1024
H, HD = 16, 64
NCORES = 8
HPC = 4  # heads per core
DPC = HPC * HD  # 256 head dims per core
KT = D // 128  # 8 contraction tiles
NSK = S // 128  # 16 s_k tiles
NJ = S // 512  # 4 q chunks
NC2 = HPC // 2  # 2 head pairs

_CACHE = {}


def _build():
    import concourse.bacc as bacc
    import concourse.mybir as mybir
    import concourse.tile as tile

    F32 = mybir.dt.float32
    BF16 = mybir.dt.bfloat16
    MULT = mybir.AluOpType.mult
    EXP = mybir.ActivationFunctionType.Exp

    nc = bacc.Bacc("TRN2", target_bir_lowering=False, debug=False)

    xC = nc.dram_tensor("xC", [NJ, 128, KT, 512], BF16, kind="ExternalInput")
    wqP = nc.dram_tensor("wqP", [128, KT, DPC], BF16, kind="ExternalInput")
    wkP = nc.dram_tensor("wkP", [128, KT, DPC], BF16, kind="ExternalInput")
    wvP = nc.dram_tensor("wvP", [128, KT, DPC], BF16, kind="ExternalInput")
    woP = nc.dram_tensor("woP", [128, 2, D], BF16, kind="ExternalInput")
    maskP = nc.dram_tensor("maskP", [128, NSK, S], BF16, kind="ExternalInput")
    outp = nc.dram_tensor("outp", [S, D], BF16, kind="ExternalOutput")

    with tile.TileContext(nc) as tc:
        with (
            tc.tile_pool(name="keep", bufs=1) as keep,
            tc.tile_pool(name="pexpt", bufs=7) as pexpt,
            tc.tile_pool(name="pexpm", bufs=8) as pexpm,
            tc.tile_pool(name="pnorm", bufs=2) as pnorm,
            tc.tile_pool(name="p3s", bufs=2) as p3s,
            tc.tile_pool(name="scp", bufs=2, space="PSUM") as scp,
            tc.tile_pool(name="opp", bufs=1, space="PSUM") as opp,
            tc.tile_pool(name="aux", bufs=2, space="PSUM") as aux,
        ):
            # ---- persistent SBUF ----------------------------------------
            x_sb = keep.tile([128, NJ, KT, 512], BF16)  # 32KB/part, j-major
            wq_sb = keep.tile([128, KT, DPC], BF16)
            wk_sb = keep.tile([128, KT, DPC], BF16)
            wv_sb = keep.tile([128, KT, DPC], BF16)
            wo_sb = keep.tile([128, 2, D], BF16)
            qT_sb = keep.tile([128, NC2, S], BF16)
            kT_sb = keep.tile([128, NC2, S], BF16)
            v_aug = keep.tile([128, NSK, HPC * 128], BF16)  # 16KB/part
            mask01 = keep.tile([128, NSK, S], BF16)  # 64KB/part, 0/1
            out_cT = keep.tile([128, NC2, S], BF16)

            nc.any.memset(v_aug[:], 1.0)

            # ---- DMAs ---------------------------------------------------
            # mask on the qAct HWDGE queue (ACT idle until the exp stream
            # ramps), everything else on qSP in need-order.
            for mp in range(8):
                nc.scalar.dma_start(
                    out=mask01[:, 2 * mp : 2 * mp + 2, :],
                    in_=maskP[:, 2 * mp : 2 * mp + 2, :],
                )

            def dma_x(jc):
                nc.sync.dma_start(out=x_sb[:, jc, :, :], in_=xC[jc, :, :, :])

            nc.sync.dma_start(out=wk_sb[:], in_=wkP[:])
            dma_x(0)
            nc.sync.dma_start(out=wq_sb[:], in_=wqP[:])
            nc.sync.dma_start(out=wv_sb[:], in_=wvP[:])
            dma_x(1)
            dma_x(2)
            dma_x(3)
            nc.sync.dma_start(out=wo_sb[:], in_=woP[:])

            # ---- projection chains (2 lazily-allocating halves) ---------
            def chain_kq(w_sb, dst_sb, c, jk):
                st = {}

                def half(r0, r1):
                    if r0 == 0:
                        st["ps"] = aux.tile(
                            [128, 512], F32, tag="aux", name=f"ch_{c}_{jk}"
                        )
                    ps = st["ps"]
                    for t in range(r0, r1):
                        nc.tensor.matmul(
                            ps[:],
                            w_sb[:, t, c * 128 : (c + 1) * 128],
                            x_sb[:, jk, t, :],
                            start=(t == 0),
                            stop=(t == KT - 1),
                        )
                    if r1 == KT:
                        sl = slice(jk * 512, (jk + 1) * 512)
                        nc.vector.tensor_copy(dst_sb[:, c, sl], ps[:])

                return [lambda: half(0, 4), lambda: half(4, KT)]

            def chain_v(sb):
                st = {}
                jv, uv = divmod(sb * 128, 512)

                def half(r0, r1):
                    if r0 == 0:
                        st["ps"] = aux.tile(
                            [128, 256], F32, tag="aux", name=f"chv_{sb}"
                        )
                    ps = st["ps"]
                    for t in range(r0, r1):
                        nc.tensor.matmul(
                            ps[:],
                            x_sb[:, jv, t, uv : uv + 128],
                            wv_sb[:, t, :],
                            start=(t == 0),
                            stop=(t == KT - 1),
                        )
                    if r1 == KT:
                        nc.vector.tensor_copy(
                            v_aug[:, sb, :]
                            .rearrange("p (h c2) -> p h c2", h=HPC)[:, :, 0:HD],
                            ps[:].rearrange("p (h c2) -> p h c2", h=HPC),
                        )

                return [lambda: half(0, 4), lambda: half(4, KT)]

            # ---- phase-3 output blocks ----------------------------------
            def phase3_pieces(j):
                """8 pieces for q chunk j: 4 m-blocks x (n=0 | n=1+DMA)."""
                pieces = []
                for m in range(4):
                    mm = j * 4 + m
                    msl = slice(mm * 128, (mm + 1) * 128)
                    ob = [None]

                    def pa(mm=mm, msl=msl, ob=ob):
                        ps = aux.tile([128, 512], F32, tag="aux", name=f"p3a_{mm}")
                        for cb in range(2):
                            nc.tensor.matmul(
                                ps[:],
                                out_cT[:, cb, msl],
                                wo_sb[:, cb, 0:512],
                                start=(cb == 0),
                                stop=(cb == 1),
                            )
                        ob[0] = p3s.tile(
                            [128, 1024], BF16, tag="ob", name=f"ob_{mm}"
                        )
                        nc.vector.tensor_copy(ob[0][:, 0:512], ps[:])

                    def pb(mm=mm, msl=msl, ob=ob):
                        ps = aux.tile([128, 512], F32, tag="aux", name=f"p3b_{mm}")
                        for cb in range(2):
                            nc.tensor.matmul(
                                ps[:],
                                out_cT[:, cb, msl],
                                wo_sb[:, cb, 512:1024],
                                start=(cb == 0),
                                stop=(cb == 1),
                            )
                        nc.vector.tensor_copy(ob[0][:, 512:1024], ps[:])
                        nc.sync.dma_start(out=outp[msl, :], in_=ob[0][:])

                    pieces += [pa, pb]
                return pieces

            # ---- the pipelined slot stream ------------------------------
            L = 6  # attnV lookahead in slots
            UNITS = [(0, 0), (1, 0), (0, 1), (1, 1), (0, 2), (1, 2), (0, 3), (1, 3)]
            NU = len(UNITS)
            NSLOT = NU * NSK

            # prefix: only what the first scores matmul needs.
            for t in chain_kq(wk_sb, kT_sb, 0, 0):
                t()
            for t in chain_kq(wq_sb, qT_sb, 0, 0):
                t()

            # extras heap: (excl_deadline, seq, ready_slot, thunk).  An
            # item must be POPPED at a slot < excl_deadline (extras are
            # emitted after scores of slot g but before attnV of g-L).
            extras = []
            _seq = [0]

            def add(deadline, pieces, ready=0):
                for p in pieces:
                    heapq.heappush(extras, (deadline, _seq[0], ready, p))
                    _seq[0] += 1

            for sb in range(NSK):
                # consumed by attnV(u=0, i=sb) at slot sb+L (after extras)
                add(sb + L + 1, chain_v(sb))
            for jk in range(1, 4):
                # consumed by scores(u=0, i=4jk) at slot 4jk (before extras)
                add(4 * jk, chain_kq(wk_sb, kT_sb, 0, jk))
            add(NSK, chain_kq(wq_sb, qT_sb, 1, 0))
            for jk in range(4):
                add(NSK + 4 * jk if jk else NSK, chain_kq(wk_sb, kT_sb, 1, jk))
            for u in range(2, NU):
                c, j = UNITS[u]
                add(u * NSK, chain_kq(wq_sb, qT_sb, c, j))
            # phase-3 for chunk j: ready once unit (1, j)'s normalization
            # multiplies are guaranteed emitted (their pop slot + 1).
            for j in range(4):
                norm_slot_1j = (2 * j + 1) * NSK + NSK - 1 + L
                add(NSLOT + L + 10, phase3_pieces(j), ready=norm_slot_1j + 2)

            out_ps = [None] * NU
            expm_ring = {}

            def emit_scores(u, i):
                c, j = UNITS[u]
                jsl = slice(j * 512, (j + 1) * 512)
                sc = scp.tile([128, 2, 512], F32, tag="sc", name=f"sc_{u}_{i}")
                for h2 in range(2):
                    hsl = slice(h2 * 64, (h2 + 1) * 64)
                    nc.tensor.matmul(
                        sc[:, h2, :],
                        kT_sb[hsl, c, i * 128 : (i + 1) * 128],
                        qT_sb[hsl, c, jsl],
                        start=True,
                        stop=True,
                    )
                expt = pexpt.tile([128, 2, 512], BF16, tag="expt", name=f"et_{u}_{i}")
                nc.scalar.activation(out=expt[:], in_=sc[:], func=EXP, scale=0.125)
                expm = pexpm.tile([128, 2, 512], BF16, tag="expm", name=f"em_{u}_{i}")
                nc.vector.tensor_tensor(
                    out=expm[:],
                    in0=expt[:],
                    in1=mask01[:, i, jsl][:, None, :].to_broadcast((128, 2, 512)),
                    op=MULT,
                )
                expm_ring[(u, i)] = expm

            def emit_attnv(u, i):
                c, j = UNITS[u]
                if i == 0:
                    out_ps[u] = opp.tile(
                        [128, 2, 512], F32, tag="ops", name=f"op_{u}"
                    )
                expm = expm_ring.pop((u, i))
                for h2 in range(2):
                    h = 2 * c + h2
                    nc.tensor.matmul(
                        out_ps[u][:, h2, :],
                        v_aug[:, i, h * 128 : (h + 1) * 128],
                        expm[:, h2, :],
                        start=(i == 0),
                        stop=(i == NSK - 1),
                    )

            def emit_norm_start(u, g):
                """Z reciprocal + gpsimd broadcast right after unit u's last
                attnV; the two DVE multiplies are spliced into the next
                slot's extras (deadline g+2: they must precede the next
                unit's first attnV, which reuses the single opp buffer)."""
                c, j = UNITS[u]
                jsl = slice(j * 512, (j + 1) * 512)
                muls = []
                for h2 in range(2):
                    zrow = pnorm.tile([1, 512], F32, tag="zrow", name=f"zw_{u}_{h2}")
                    nc.vector.tensor_copy(zrow[:], out_ps[u][64:65, h2, :])
                    zr1 = pnorm.tile([1, 512], F32, tag="zr1", name=f"z1_{u}_{h2}")
                    nc.vector.reciprocal_approx_fast(out=zr1[:], in_=zrow[:])
                    zr = pnorm.tile([64, 512], F32, tag="zr", name=f"zr_{u}_{h2}")
                    nc.gpsimd.partition_broadcast(zr[:], zr1[:])

                    def mul(u=u, c=c, h2=h2, jsl=jsl, zr=zr):
                        nc.vector.tensor_tensor(
                            out=out_cT[h2 * 64 : (h2 + 1) * 64, c, jsl],
                            in0=out_ps[u][0:64, h2, :],
                            in1=zr[:],
                            op=MULT,
                        )

                    muls.append(mul)
                add(g + 2, muls, ready=g + 1)

            norm_slot = {u * NSK + NSK - 1 + L: u for u in range(NU)}

            def pop_extras(g):
                popped = 0
                deferred = []
                while extras:
                    dl, sq, ready, th = heapq.heappop(extras)
                    if ready > g:
                        deferred.append((dl, sq, ready, th))
                        continue
                    assert dl > g, (
                        f"extras deadline miss: item dl={dl} seq={sq} at slot {g}"
                    )
                    cap = 3 if dl <= g + 2 else (2 if dl <= g + 6 else 1)
                    if popped >= cap:
                        deferred.append((dl, sq, ready, th))
                        break
                    th()
                    popped += 1
                for item in deferred:
                    heapq.heappush(extras, item)

            for g in range(NSLOT + L + 1):
                u, i = divmod(g, NSK)
                if u < NU:
                    emit_scores(u, i)
                pop_extras(g)
                g2 = g - L
                if 0 <= g2 < NSLOT:
                    emit_attnv(*divmod(g2, NSK))
                if g in norm_slot:
                    emit_norm_start(norm_slot[g], g)

            # drain (deadline order keeps unit-7 norm muls before phase3(3))
            while extras:
                dl, sq, ready, th = heapq.heappop(extras)
                th()

    nc.compile()
    return nc


def _get_nc():
    if "nc" not in _CACHE:
        _CACHE["nc"] = _build()
    return _CACHE["nc"]


def _prep_inputs(x, mask, Wq, Wk, Wv, Wo, bo):
    """Build the 8 per-core input maps (bf16 on the wire, p-major)."""
    import ml_dtypes

    bf16 = ml_dtypes.bfloat16
    x = np.asarray(x, dtype=np.float32)
    mask = np.asarray(mask, dtype=np.int32)
    wqT = np.asarray(Wq, np.float32).T
    wkT = np.asarray(Wk, np.float32).T
    wvT = np.asarray(Wv, np.float32).T
    woT = np.asarray(Wo, np.float32).T

    # x[b].T chunked: [NJ, 128, KT, 512] with xC[j, p, t, u] =
    # x[b].T[t*128+p, j*512+u]  (8KB per-partition contiguous runs)
    xCs = []
    for b in range(B):
        xT = x[b].T.astype(bf16)  # [D, S]
        xc = np.ascontiguousarray(
            xT.reshape(KT, 128, NJ, 512).transpose(2, 1, 0, 3)
        )
        xCs.append(xc)
    # keep-mask p-major: maskP[p, i, q] = (mask[b,0,q,i*128+p] == 0)
    maskPs = []
    for b in range(B):
        keepT = (mask[b, 0].T == 0).astype(bf16)  # [k, q]
        maskPs.append(
            np.ascontiguousarray(keepT.reshape(NSK, 128, S).transpose(1, 0, 2))
        )

    def wpm(wT, doff):  # [D, DPC] slice -> [128, KT, DPC]
        sl = np.ascontiguousarray(wT[:, doff : doff + DPC]).astype(bf16)
        return np.ascontiguousarray(sl.reshape(KT, 128, DPC).transpose(1, 0, 2))

    in_maps = []
    for c in range(NCORES):
        b, hq = c >> 2, c & 3
        doff = hq * DPC
        wos = np.ascontiguousarray(woT[doff : doff + DPC, :]).astype(bf16)
        in_maps.append(
            {
                "xC": xCs[b],
                "wqP": wpm(wqT, doff),
                "wkP": wpm(wkT, doff),
                "wvP": wpm(wvT, doff),
                "woP": np.ascontiguousarray(
                    wos.reshape(2, 128, D).transpose(1, 0, 2)
                ),
                "maskP": maskPs[b],
            }
        )
    return in_maps


def run(inputs: dict, trace: bool = False):
    """Run the kernel; returns (full_output, BassKernelResults)."""
    from concourse.bass_utils import run_bass_kernel_spmd

    nc = _get_nc()
    in_maps = _prep_inputs(**inputs)
    res = run_bass_kernel_spmd(
        nc, in_maps, core_ids=list(range(NCORES)), trace=trace
    )
    bo = np.asarray(inputs["bo"], dtype=np.float32)
    out = np.empty((B, S, D), dtype=np.float32)
    for b in range(B):
        acc = res.results[4 * b]["outp"].astype(np.float32)
        for hq in range(1, 4):
            acc = acc + res.results[4 * b + hq]["outp"].astype(np.float32)
        out[b] = acc + bo[None, :]
    return out, res


def kernel(**inputs) -> np.ndarray:
    out, _ = run(inputs, trace=False)
    return out


# revision 7
# speedup vs baseline: 1.0082x; 1.0082x over previous
"""Multi-head attention (B=2, S=2048, D=1024, H=16, Hd=64) on 8 Trainium2
NeuronCores.

Sharding: 8 cores = (batch 2) x (head-quarter 4).  Core (b, hq) computes,
for batch b and heads hq*4..hq*4+3, the full-sequence partial output

    outp = (softmax-attention of its 4 heads over all 2048 q rows) @ Wo_part.T

and the host sums the four head-quarter partials per batch and adds bo.

Everything is bf16 on the wire and in SBUF (PSUM accumulates fp32).
Host-side layouts are all p-major so every DMA descriptor is >= 4KB:
  xC     [NJ, 128, KT, 512]   x[b].T j-chunked, 8KB runs
  w{q,k,v}P [128, KT, DPC]    W.T slice, 4KB runs
  woP    [128, 2, D]          Wo.T slice, 4KB runs
  maskP  [128, NSK, S]        keep-mask (mask[b,0]==0).T, 4KB runs/tile
  outp   [S, D]               bf16 partial, 2KB rows

DMA queues: the 8MB mask streams on the qAct HWDGE queue (issued while
ACT is still idle) in parallel with x/weights/outputs on qSP.

Execution is one software-pipelined slot stream over all 8 (c, j) units
(c = head pair, j = 512-wide q chunk), 16 s_k tiles each — no
per-unit pipeline drain.  Slot g: scores matmul pair -> exp (ScalarE,
the pacing engine) -> keep-mask multiply (VectorE, 2x bf16 mode);
attnV accumulates L slots behind.  Projection chains, softmax-
normalization multiplies, and phase-3 output blocks are emitted from a
deadline-ordered (EDF) work heap, at most a few sub-microsecond pieces
per slot, so the PE stays dense without stalling the exp stream.
Emission order IS dependency order for the tile framework, so every
piece carries an exclusive deadline (= first slot that consumes it)
checked at build time.

Head packing: a pair's two heads live on partition halves 0-63/64-127
so the pair's two scores matmuls run concurrently on disjoint PE row
groups.  V lands in v_aug [128, NSK, head*128 + (64 V | 64 ones)]; the
ones columns make the attnV matmul accumulate Z = sum(expm) into PSUM
rows 64..127 at no extra stream cost (the stream time is set by the
512 moving columns, not the stationary width).
"""

import sys

if "/opt/trn_rl_repo" not in sys.path:
    sys.path.insert(0, "/opt/trn_rl_repo")

import heapq

import numpy as np

B, S, D = 2, 2048, 1024
H, HD = 16, 64
NCORES = 8
HPC = 4  # heads per core
DPC = HPC * HD  # 256 head dims per core
KT = D // 128  # 8 contraction tiles
NSK = S // 128  # 16 s_k tiles
NJ = S // 512  # 4 q chunks
NC2 = HPC // 2  # 2 head pairs

_CACHE = {}


def _build():
    import concourse.bacc as bacc
    import concourse.mybir as mybir
    import concourse.tile as tile

    F32 = mybir.dt.float32
    BF16 = mybir.dt.bfloat16
    MULT = mybir.AluOpType.mult
    EXP = mybir.ActivationFunctionType.Exp

    nc = bacc.Bacc("TRN2", target_bir_lowering=False, debug=False)

    xC = nc.dram_tensor("xC", [NJ, 128, KT, 512], BF16, kind="ExternalInput")
    wqP = nc.dram_tensor("wqP", [128, KT, DPC], BF16, kind="ExternalInput")
    wkP = nc.dram_tensor("wkP", [128, KT, DPC], BF16, kind="ExternalInput")
    wvP = nc.dram_tensor("wvP", [128, KT, DPC], BF16, kind="ExternalInput")
    woP = nc.dram_tensor("woP", [128, 2, D], BF16, kind="ExternalInput")
    maskP = nc.dram_tensor("maskP", [128, NSK, S], BF16, kind="ExternalInput")
    outp = nc.dram_tensor("outp", [S, D], BF16, kind="ExternalOutput")

    with tile.TileContext(nc) as tc:
        with (
            tc.tile_pool(name="keep", bufs=1) as keep,
            tc.tile_pool(name="pexpt", bufs=7) as pexpt,
            tc.tile_pool(name="pexpm", bufs=8) as pexpm,
            tc.tile_pool(name="pnorm", bufs=2) as pnorm,
            tc.tile_pool(name="p3s", bufs=2) as p3s,
            tc.tile_pool(name="scp", bufs=2, space="PSUM") as scp,
            tc.tile_pool(name="opp", bufs=1, space="PSUM") as opp,
            tc.tile_pool(name="aux", bufs=2, space="PSUM") as aux,
        ):
            # ---- persistent SBUF ----------------------------------------
            x_sb = keep.tile([128, NJ, KT, 512], BF16)  # 32KB/part, j-major
            wq_sb = keep.tile([128, KT, DPC], BF16)
            wk_sb = keep.tile([128, KT, DPC], BF16)
            wv_sb = keep.tile([128, KT, DPC], BF16)
            wo_sb = keep.tile([128, 2, D], BF16)
            qT_sb = keep.tile([128, NC2, S], BF16)
            kT_sb = keep.tile([128, NC2, S], BF16)
            v_aug = keep.tile([128, NSK, HPC * 128], BF16)  # 16KB/part
            mask01 = keep.tile([128, NSK, S], BF16)  # 64KB/part, 0/1
            out_cT = keep.tile([128, NC2, S], BF16)

            nc.any.memset(v_aug[:], 1.0)

            # ---- DMAs ---------------------------------------------------
            # Mask tiles 0-7 go as TWO chunks on the qAct HWDGE queue (the
            # ring keeps only ~2 DMAs in flight; a third issue would block
            # the ACT sequencer and delay the exp stream).  Tiles 8-15 ride
            # the qSP queue interleaved with the x chunks in need-order.
            def dma_mask(engine, lo, hi):
                engine.dma_start(
                    out=mask01[:, lo:hi, :], in_=maskP[:, lo:hi, :]
                )

            dma_mask(nc.scalar, 0, 4)
            dma_mask(nc.scalar, 4, 8)

            def dma_x(jc):
                nc.sync.dma_start(out=x_sb[:, jc, :, :], in_=xC[jc, :, :, :])

            nc.sync.dma_start(out=wk_sb[:], in_=wkP[:])
            dma_x(0)
            nc.sync.dma_start(out=wq_sb[:], in_=wqP[:])
            nc.sync.dma_start(out=wv_sb[:], in_=wvP[:])
            dma_x(1)
            dma_x(2)
            dma_mask(nc.sync, 8, 10)
            dma_x(3)
            dma_mask(nc.sync, 10, 12)
            dma_mask(nc.sync, 12, 14)
            dma_mask(nc.sync, 14, 16)
            nc.sync.dma_start(out=wo_sb[:], in_=woP[:])

            # ---- projection chains (2 lazily-allocating halves) ---------
            def chain_kq(w_sb, dst_sb, c, jk):
                st = {}

                def half(r0, r1):
                    if r0 == 0:
                        st["ps"] = aux.tile(
                            [128, 512], F32, tag="aux", name=f"ch_{c}_{jk}"
                        )
                    ps = st["ps"]
                    for t in range(r0, r1):
                        nc.tensor.matmul(
                            ps[:],
                            w_sb[:, t, c * 128 : (c + 1) * 128],
                            x_sb[:, jk, t, :],
                            start=(t == 0),
                            stop=(t == KT - 1),
                        )
                    if r1 == KT:
                        sl = slice(jk * 512, (jk + 1) * 512)
                        nc.vector.tensor_copy(dst_sb[:, c, sl], ps[:])

                return [lambda: half(0, 4), lambda: half(4, KT)]

            def chain_v(sb):
                st = {}
                jv, uv = divmod(sb * 128, 512)

                def half(r0, r1):
                    if r0 == 0:
                        st["ps"] = aux.tile(
                            [128, 256], F32, tag="aux", name=f"chv_{sb}"
                        )
                    ps = st["ps"]
                    for t in range(r0, r1):
                        nc.tensor.matmul(
                            ps[:],
                            x_sb[:, jv, t, uv : uv + 128],
                            wv_sb[:, t, :],
                            start=(t == 0),
                            stop=(t == KT - 1),
                        )
                    if r1 == KT:
                        nc.vector.tensor_copy(
                            v_aug[:, sb, :]
                            .rearrange("p (h c2) -> p h c2", h=HPC)[:, :, 0:HD],
                            ps[:].rearrange("p (h c2) -> p h c2", h=HPC),
                        )

                return [lambda: half(0, 4), lambda: half(4, KT)]

            # ---- phase-3 output blocks ----------------------------------
            def phase3_pieces(j):
                """8 pieces for q chunk j: 4 m-blocks x (n=0 | n=1+DMA)."""
                pieces = []
                for m in range(4):
                    mm = j * 4 + m
                    msl = slice(mm * 128, (mm + 1) * 128)
                    ob = [None]

                    def pa(mm=mm, msl=msl, ob=ob):
                        ps = aux.tile([128, 512], F32, tag="aux", name=f"p3a_{mm}")
                        for cb in range(2):
                            nc.tensor.matmul(
                                ps[:],
                                out_cT[:, cb, msl],
                                wo_sb[:, cb, 0:512],
                                start=(cb == 0),
                                stop=(cb == 1),
                            )
                        ob[0] = p3s.tile(
                            [128, 1024], BF16, tag="ob", name=f"ob_{mm}"
                        )
                        nc.vector.tensor_copy(ob[0][:, 0:512], ps[:])

                    def pb(mm=mm, msl=msl, ob=ob):
                        ps = aux.tile([128, 512], F32, tag="aux", name=f"p3b_{mm}")
                        for cb in range(2):
                            nc.tensor.matmul(
                                ps[:],
                                out_cT[:, cb, msl],
                                wo_sb[:, cb, 512:1024],
                                start=(cb == 0),
                                stop=(cb == 1),
                            )
                        nc.vector.tensor_copy(ob[0][:, 512:1024], ps[:])
                        nc.sync.dma_start(out=outp[msl, :], in_=ob[0][:])

                    pieces += [pa, pb]
                return pieces

            # ---- the pipelined slot stream ------------------------------
            L = 6  # attnV lookahead in slots
            UNITS = [(0, 0), (1, 0), (0, 1), (1, 1), (0, 2), (1, 2), (0, 3), (1, 3)]
            NU = len(UNITS)
            NSLOT = NU * NSK

            # prefix: only what the first scores matmul needs.
            for t in chain_kq(wk_sb, kT_sb, 0, 0):
                t()
            for t in chain_kq(wq_sb, qT_sb, 0, 0):
                t()

            # extras heap: (sched_dl, seq, ready_slot, true_dl, thunk).  An
            # item must be POPPED at a slot < true_dl (extras are emitted
            # after scores of slot g but before attnV of g-L).  sched_dl
            # runs SLACK slots earlier so EDF emits producers well before
            # their consumers instead of maximally late (a just-in-time
            # chain leaves its consumer stalled on the chain's execution).
            extras = []
            _seq = [0]
            SLACK = 6

            def add(deadline, pieces, ready=0, slack=SLACK):
                for p in pieces:
                    heapq.heappush(
                        extras,
                        (max(deadline - slack, 0), _seq[0], ready, deadline, p),
                    )
                    _seq[0] += 1

            for sb in range(NSK):
                # consumed by attnV(u=0, i=sb) at slot sb+L (after extras)
                add(sb + L + 1, chain_v(sb))
            for jk in range(1, 4):
                # consumed by scores(u=0, i=4jk) at slot 4jk (before extras)
                add(4 * jk, chain_kq(wk_sb, kT_sb, 0, jk))
            add(NSK, chain_kq(wq_sb, qT_sb, 1, 0))
            for jk in range(4):
                add(NSK + 4 * jk if jk else NSK, chain_kq(wk_sb, kT_sb, 1, jk))
            for u in range(2, NU):
                c, j = UNITS[u]
                add(u * NSK, chain_kq(wq_sb, qT_sb, c, j))
            # phase-3 for chunk j: ready once unit (1, j)'s normalization
            # multiplies are guaranteed emitted (their pop slot + 1).
            for j in range(4):
                norm_slot_1j = (2 * j + 1) * NSK + NSK - 1 + L
                add(NSLOT + L + 10, phase3_pieces(j), ready=norm_slot_1j + 2)

            out_ps = [None] * NU
            expm_ring = {}

            def emit_scores(u, i):
                c, j = UNITS[u]
                jsl = slice(j * 512, (j + 1) * 512)
                sc = scp.tile([128, 2, 512], F32, tag="sc", name=f"sc_{u}_{i}")
                for h2 in range(2):
                    hsl = slice(h2 * 64, (h2 + 1) * 64)
                    nc.tensor.matmul(
                        sc[:, h2, :],
                        kT_sb[hsl, c, i * 128 : (i + 1) * 128],
                        qT_sb[hsl, c, jsl],
                        start=True,
                        stop=True,
                    )
                expt = pexpt.tile([128, 2, 512], BF16, tag="expt", name=f"et_{u}_{i}")
                nc.scalar.activation(out=expt[:], in_=sc[:], func=EXP, scale=0.125)
                expm = pexpm.tile([128, 2, 512], BF16, tag="expm", name=f"em_{u}_{i}")
                nc.vector.tensor_tensor(
                    out=expm[:],
                    in0=expt[:],
                    in1=mask01[:, i, jsl][:, None, :].to_broadcast((128, 2, 512)),
                    op=MULT,
                )
                expm_ring[(u, i)] = expm

            def emit_attnv(u, i):
                c, j = UNITS[u]
                if i == 0:
                    out_ps[u] = opp.tile(
                        [128, 2, 512], F32, tag="ops", name=f"op_{u}"
                    )
                expm = expm_ring.pop((u, i))
                for h2 in range(2):
                    h = 2 * c + h2
                    nc.tensor.matmul(
                        out_ps[u][:, h2, :],
                        v_aug[:, i, h * 128 : (h + 1) * 128],
                        expm[:, h2, :],
                        start=(i == 0),
                        stop=(i == NSK - 1),
                    )

            def emit_norm_start(u, g):
                """Z reciprocal + gpsimd broadcast right after unit u's last
                attnV; the two DVE multiplies are spliced into the next
                slot's extras (deadline g+2: they must precede the next
                unit's first attnV, which reuses the single opp buffer)."""
                c, j = UNITS[u]
                jsl = slice(j * 512, (j + 1) * 512)
                muls = []
                for h2 in range(2):
                    zrow = pnorm.tile([1, 512], F32, tag="zrow", name=f"zw_{u}_{h2}")
                    nc.vector.tensor_copy(zrow[:], out_ps[u][64:65, h2, :])
                    zr1 = pnorm.tile([1, 512], F32, tag="zr1", name=f"z1_{u}_{h2}")
                    nc.vector.reciprocal_approx_fast(out=zr1[:], in_=zrow[:])
                    zr = pnorm.tile([64, 512], F32, tag="zr", name=f"zr_{u}_{h2}")
                    nc.gpsimd.partition_broadcast(zr[:], zr1[:])

                    def mul(u=u, c=c, h2=h2, jsl=jsl, zr=zr):
                        nc.vector.tensor_tensor(
                            out=out_cT[h2 * 64 : (h2 + 1) * 64, c, jsl],
                            in0=out_ps[u][0:64, h2, :],
                            in1=zr[:],
                            op=MULT,
                        )

                    muls.append(mul)
                add(g + 2, muls, ready=g + 1)

            norm_slot = {u * NSK + NSK - 1 + L: u for u in range(NU)}

            def pop_extras(g):
                popped = 0
                deferred = []
                while extras:
                    item = heapq.heappop(extras)
                    dl, sq, ready, true_dl, th = item
                    if ready > g:
                        deferred.append(item)
                        continue
                    assert true_dl > g, (
                        f"extras deadline miss: dl={true_dl} seq={sq} slot={g}"
                    )
                    cap = 3 if dl <= g + 2 else (2 if dl <= g + 6 else 1)
                    if popped >= cap:
                        deferred.append(item)
                        break
                    th()
                    popped += 1
                for item in deferred:
                    heapq.heappush(extras, item)

            for g in range(NSLOT + L + 1):
                u, i = divmod(g, NSK)
                if u < NU:
                    emit_scores(u, i)
                pop_extras(g)
                g2 = g - L
                if 0 <= g2 < NSLOT:
                    emit_attnv(*divmod(g2, NSK))
                if g in norm_slot:
                    emit_norm_start(norm_slot[g], g)

            # drain (deadline order keeps unit-7 norm muls before phase3(3))
            while extras:
                dl, sq, ready, true_dl, th = heapq.heappop(extras)
                th()

    nc.compile()
    return nc


def _get_nc():
    if "nc" not in _CACHE:
        _CACHE["nc"] = _build()
    return _CACHE["nc"]


def _prep_inputs(x, mask, Wq, Wk, Wv, Wo, bo):
    """Build the 8 per-core input maps (bf16 on the wire, p-major)."""
    import ml_dtypes

    bf16 = ml_dtypes.bfloat16
    x = np.asarray(x, dtype=np.float32)
    mask = np.asarray(mask, dtype=np.int32)
    wqT = np.asarray(Wq, np.float32).T
    wkT = np.asarray(Wk, np.float32).T
    wvT = np.asarray(Wv, np.float32).T
    woT = np.asarray(Wo, np.float32).T

    # x[b].T chunked: [NJ, 128, KT, 512] with xC[j, p, t, u] =
    # x[b].T[t*128+p, j*512+u]  (8KB per-partition contiguous runs)
    xCs = []
    for b in range(B):
        xT = x[b].T.astype(bf16)  # [D, S]
        xc = np.ascontiguousarray(
            xT.reshape(KT, 128, NJ, 512).transpose(2, 1, 0, 3)
        )
        xCs.append(xc)
    # keep-mask p-major: maskP[p, i, q] = (mask[b,0,q,i*128+p] == 0)
    maskPs = []
    for b in range(B):
        keepT = (mask[b, 0].T == 0).astype(bf16)  # [k, q]
        maskPs.append(
            np.ascontiguousarray(keepT.reshape(NSK, 128, S).transpose(1, 0, 2))
        )

    def wpm(wT, doff):  # [D, DPC] slice -> [128, KT, DPC]
        sl = np.ascontiguousarray(wT[:, doff : doff + DPC]).astype(bf16)
        return np.ascontiguousarray(sl.reshape(KT, 128, DPC).transpose(1, 0, 2))

    in_maps = []
    for c in range(NCORES):
        b, hq = c >> 2, c & 3
        doff = hq * DPC
        wos = np.ascontiguousarray(woT[doff : doff + DPC, :]).astype(bf16)
        in_maps.append(
            {
                "xC": xCs[b],
                "wqP": wpm(wqT, doff),
                "wkP": wpm(wkT, doff),
                "wvP": wpm(wvT, doff),
                "woP": np.ascontiguousarray(
                    wos.reshape(2, 128, D).transpose(1, 0, 2)
                ),
                "maskP": maskPs[b],
            }
        )
    return in_maps


def run(inputs: dict, trace: bool = False):
    """Run the kernel; returns (full_output, BassKernelResults)."""
    from concourse.bass_utils import run_bass_kernel_spmd

    nc = _get_nc()
    in_maps = _prep_inputs(**inputs)
    res = run_bass_kernel_spmd(
        nc, in_maps, core_ids=list(range(NCORES)), trace=trace
    )
    bo = np.asarray(inputs["bo"], dtype=np.float32)
    out = np.empty((B, S, D), dtype=np.float32)
    for b in range(B):
        acc = res.results[4 * b]["outp"].astype(np.float32)
        for hq in range(1, 4):
            acc = acc + res.results[4 * b + hq]["outp"].astype(np.float32)
        out[b] = acc + bo[None, :]
    return out, res


def kernel(**inputs) -> np.ndarray:
    out, _ = run(inputs, trace=False)
    return out


# revision 11
# speedup vs baseline: 1.0387x; 1.0303x over previous
"""Multi-head attention (B=2, S=2048, D=1024, H=16, Hd=64) on 8 Trainium2
NeuronCores.

Sharding: 8 cores = (batch 2) x (head-quarter 4).  Core (b, hq) computes,
for batch b and heads hq*4..hq*4+3, the full-sequence partial output

    outp = (softmax-attention of its 4 heads over all 2048 q rows) @ Wo_part.T

and the host sums the four head-quarter partials per batch and adds bo.

Everything is bf16 on the wire and in SBUF (PSUM accumulates fp32).
Host-side layouts are all p-major so every DMA descriptor is >= 4KB:
  xC     [NJ, 128, KT, 512]   x[b].T j-chunked, 8KB runs
  w{q,k,v}P [128, KT, DPC]    W.T slice, 4KB runs
  woP    [128, 2, D]          Wo.T slice, 4KB runs
  maskP  [128, NSK, S]        keep-mask (mask[b,0]==0).T, 4KB runs/tile
  outp   [S, D]               bf16 partial, 2KB rows

DMA queues: the 8MB mask streams on the qAct HWDGE queue (issued while
ACT is still idle) in parallel with x/weights/outputs on qSP.

Execution is one software-pipelined slot stream over all 8 (c, j) units
(c = head pair, j = 512-wide q chunk), 16 s_k tiles each — no
per-unit pipeline drain.  Slot g: scores matmul pair -> exp (ScalarE,
the pacing engine) -> keep-mask multiply (VectorE, 2x bf16 mode);
attnV accumulates L slots behind.  Projection chains, softmax-
normalization multiplies, and phase-3 output blocks are emitted from a
deadline-ordered (EDF) work heap, at most a few sub-microsecond pieces
per slot, so the PE stays dense without stalling the exp stream.
Emission order IS dependency order for the tile framework, so every
piece carries an exclusive deadline (= first slot that consumes it)
checked at build time.

Head packing: a pair's two heads live on partition halves 0-63/64-127
so the pair's two scores matmuls run concurrently on disjoint PE row
groups.  V lands in v_aug [128, NSK, head*128 + (64 V | 64 ones)]; the
ones columns make the attnV matmul accumulate Z = sum(expm) into PSUM
rows 64..127 at no extra stream cost (the stream time is set by the
512 moving columns, not the stationary width).
"""

import sys

if "/opt/trn_rl_repo" not in sys.path:
    sys.path.insert(0, "/opt/trn_rl_repo")

import heapq

import numpy as np

B, S, D = 2, 2048, 1024
H, HD = 16, 64
NCORES = 8
HPC = 4  # heads per core
DPC = HPC * HD  # 256 head dims per core
KT = D // 128  # 8 contraction tiles
NSK = S // 128  # 16 s_k tiles
NJ = S // 512  # 4 q chunks
NC2 = HPC // 2  # 2 head pairs

_CACHE = {}


def _build():
    import concourse.bacc as bacc
    import concourse.mybir as mybir
    import concourse.tile as tile

    F32 = mybir.dt.float32
    BF16 = mybir.dt.bfloat16
    MULT = mybir.AluOpType.mult
    EXP = mybir.ActivationFunctionType.Exp

    nc = bacc.Bacc("TRN2", target_bir_lowering=False, debug=False)

    xC = nc.dram_tensor("xC", [NJ, 128, KT, 512], BF16, kind="ExternalInput")
    wqP = nc.dram_tensor("wqP", [128, KT, DPC], BF16, kind="ExternalInput")
    wkP = nc.dram_tensor("wkP", [128, KT, DPC], BF16, kind="ExternalInput")
    wvP = nc.dram_tensor("wvP", [128, KT, DPC], BF16, kind="ExternalInput")
    woP = nc.dram_tensor("woP", [128, 2, D], BF16, kind="ExternalInput")
    maskP = nc.dram_tensor("maskP", [128, NSK, S], BF16, kind="ExternalInput")
    outp = nc.dram_tensor("outp", [S, D], BF16, kind="ExternalOutput")

    with tile.TileContext(nc) as tc:
        with (
            tc.tile_pool(name="keep", bufs=1) as keep,
            tc.tile_pool(name="pexpt", bufs=7) as pexpt,
            tc.tile_pool(name="pexpm", bufs=8) as pexpm,
            tc.tile_pool(name="pnorm", bufs=2) as pnorm,
            tc.tile_pool(name="p3s", bufs=2) as p3s,
            tc.tile_pool(name="scp", bufs=2, space="PSUM") as scp,
            tc.tile_pool(name="opp", bufs=1, space="PSUM") as opp,
            tc.tile_pool(name="aux", bufs=2, space="PSUM") as aux,
        ):
            # ---- persistent SBUF ----------------------------------------
            x_sb = keep.tile([128, NJ, KT, 512], BF16)  # 32KB/part, j-major
            wq_sb = keep.tile([128, KT, DPC], BF16)
            wk_sb = keep.tile([128, KT, DPC], BF16)
            wv_sb = keep.tile([128, KT, DPC], BF16)
            wo_sb = keep.tile([128, 2, D], BF16)
            qT_sb = keep.tile([128, NC2, S], BF16)
            kT_sb = keep.tile([128, NC2, S], BF16)
            v_aug = keep.tile([128, NSK, HPC * 128], BF16)  # 16KB/part
            mask01 = keep.tile([128, NSK, S], BF16)  # 64KB/part, 0/1
            out_cT = keep.tile([128, NC2, S], BF16)

            nc.any.memset(v_aug[:], 1.0)

            # ---- DMAs ---------------------------------------------------
            # Mask tiles 0-7 go as TWO chunks on the qAct HWDGE queue (the
            # ring keeps only ~2 DMAs in flight; a third issue would block
            # the ACT sequencer and delay the exp stream).  Tiles 8-15 ride
            # the qSP queue interleaved with the x chunks in need-order.
            def dma_mask(engine, lo, hi):
                engine.dma_start(
                    out=mask01[:, lo:hi, :], in_=maskP[:, lo:hi, :]
                )

            dma_mask(nc.scalar, 0, 4)
            dma_mask(nc.scalar, 4, 8)

            def dma_x(jc):
                nc.sync.dma_start(out=x_sb[:, jc, :, :], in_=xC[jc, :, :, :])

            nc.sync.dma_start(out=wk_sb[:], in_=wkP[:])
            dma_x(0)
            nc.sync.dma_start(out=wq_sb[:], in_=wqP[:])
            nc.sync.dma_start(out=wv_sb[:], in_=wvP[:])
            dma_x(1)
            dma_x(2)
            dma_mask(nc.sync, 8, 10)
            dma_x(3)
            dma_mask(nc.sync, 10, 12)
            dma_mask(nc.sync, 12, 14)
            dma_mask(nc.sync, 14, 16)
            nc.sync.dma_start(out=wo_sb[:], in_=woP[:])

            # ---- projection chains --------------------------------------
            def chain_kq(w_sb, dst_sb, c, jk, npieces=4):
                st = {}
                bounds = [
                    (KT * p // npieces, KT * (p + 1) // npieces)
                    for p in range(npieces)
                ]

                def piece(r0, r1):
                    if r0 == 0:
                        st["ps"] = aux.tile(
                            [128, 512], F32, tag="aux", name=f"ch_{c}_{jk}"
                        )
                    ps = st["ps"]
                    for t in range(r0, r1):
                        nc.tensor.matmul(
                            ps[:],
                            w_sb[:, t, c * 128 : (c + 1) * 128],
                            x_sb[:, jk, t, :],
                            start=(t == 0),
                            stop=(t == KT - 1),
                        )
                    if r1 == KT:
                        sl = slice(jk * 512, (jk + 1) * 512)
                        nc.vector.tensor_copy(dst_sb[:, c, sl], ps[:])

                return [lambda r0=r0, r1=r1: piece(r0, r1) for r0, r1 in bounds]

            def chain_v(sb):
                st = {}
                jv, uv = divmod(sb * 128, 512)

                def half(r0, r1):
                    if r0 == 0:
                        st["ps"] = aux.tile(
                            [128, 256], F32, tag="aux", name=f"chv_{sb}"
                        )
                    ps = st["ps"]
                    for t in range(r0, r1):
                        nc.tensor.matmul(
                            ps[:],
                            x_sb[:, jv, t, uv : uv + 128],
                            wv_sb[:, t, :],
                            start=(t == 0),
                            stop=(t == KT - 1),
                        )
                    if r1 == KT:
                        nc.vector.tensor_copy(
                            v_aug[:, sb, :]
                            .rearrange("p (h c2) -> p h c2", h=HPC)[:, :, 0:HD],
                            ps[:].rearrange("p (h c2) -> p h c2", h=HPC),
                        )

                return [lambda: half(0, 4), lambda: half(4, KT)]

            # ---- phase-3 output blocks ----------------------------------
            def phase3_pieces(j):
                """8 pieces for q chunk j: 4 m-blocks x (n=0 | n=1+DMA)."""
                pieces = []
                for m in range(4):
                    mm = j * 4 + m
                    msl = slice(mm * 128, (mm + 1) * 128)
                    ob = [None]

                    def pa(mm=mm, msl=msl, ob=ob):
                        ps = aux.tile([128, 512], F32, tag="aux", name=f"p3a_{mm}")
                        for cb in range(2):
                            nc.tensor.matmul(
                                ps[:],
                                out_cT[:, cb, msl],
                                wo_sb[:, cb, 0:512],
                                start=(cb == 0),
                                stop=(cb == 1),
                            )
                        ob[0] = p3s.tile(
                            [128, 1024], BF16, tag="ob", name=f"ob_{mm}"
                        )
                        nc.vector.tensor_copy(ob[0][:, 0:512], ps[:])

                    def pb(mm=mm, msl=msl, ob=ob):
                        ps = aux.tile([128, 512], F32, tag="aux", name=f"p3b_{mm}")
                        for cb in range(2):
                            nc.tensor.matmul(
                                ps[:],
                                out_cT[:, cb, msl],
                                wo_sb[:, cb, 512:1024],
                                start=(cb == 0),
                                stop=(cb == 1),
                            )
                        nc.vector.tensor_copy(ob[0][:, 512:1024], ps[:])
                        nc.sync.dma_start(out=outp[msl, :], in_=ob[0][:])

                    pieces += [pa, pb]
                return pieces

            # ---- the pipelined slot stream ------------------------------
            # c-major unit order: units 0-3 need only prefix chains, so
            # in-stream extras stay light and the PE keeps its clock up.
            L = 6  # attnV lookahead in slots
            UNITS = [(0, 0), (0, 1), (0, 2), (0, 3), (1, 0), (1, 1), (1, 2), (1, 3)]
            NU = len(UNITS)
            NSLOT = NU * NSK

            # prefix: everything c=0 + all of V, back-to-back (the PE ramps
            # to full clock on an uninterrupted run; unit 0's exp stream is
            # mask-DMA paced anyway, so a later stream start costs nothing).
            prefix = (
                chain_kq(wk_sb, kT_sb, 0, 0, 1)
                + chain_kq(wq_sb, qT_sb, 0, 0, 1)
                + chain_v(0) + chain_v(1) + chain_v(2) + chain_v(3)
                + chain_kq(wk_sb, kT_sb, 0, 1, 1)
                + chain_v(4) + chain_v(5) + chain_v(6) + chain_v(7)
                + chain_kq(wk_sb, kT_sb, 0, 2, 1)
                + chain_v(8) + chain_v(9) + chain_v(10) + chain_v(11)
                + chain_kq(wk_sb, kT_sb, 0, 3, 1)
                + chain_v(12) + chain_v(13) + chain_v(14) + chain_v(15)
            )
            for t in prefix:
                t()

            # extras heap: (sched_dl, seq, ready_slot, true_dl, thunk).  An
            # item must be POPPED at a slot < true_dl (extras are emitted
            # after scores of slot g but before attnV of g-L).  sched_dl
            # runs SLACK slots earlier so EDF emits producers well before
            # their consumers instead of maximally late (a just-in-time
            # chain leaves its consumer stalled on the chain's execution).
            extras = []
            _seq = [0]
            SLACK = 6

            def add(deadline, pieces, ready=0, slack=SLACK):
                for p in pieces:
                    heapq.heappush(
                        extras,
                        (max(deadline - slack, 0), _seq[0], ready, deadline, p),
                    )
                    _seq[0] += 1

            # Q(0,j) consumed by unit j's scores at slot 16j; K(1,jk) and
            # Q(1,j) by units 4-7.  All as quarter pieces (~2 matmuls) so
            # no chain ever bursts the PE mid-stream.
            for j in range(1, 4):
                add(NSK * j, chain_kq(wq_sb, qT_sb, 0, j))
            add(4 * NSK, chain_kq(wq_sb, qT_sb, 1, 0))
            for jk in range(4):
                add(4 * NSK + 4 * jk, chain_kq(wk_sb, kT_sb, 1, jk))
            for j in range(1, 4):
                add((4 + j) * NSK, chain_kq(wq_sb, qT_sb, 1, j))
            # phase-3 for chunk j: ready once unit (1, j)'s (= unit 4+j)
            # normalization multiplies are guaranteed emitted.
            for j in range(4):
                norm_slot_1j = (4 + j) * NSK + NSK - 1 + L
                add(NSLOT + L + 10, phase3_pieces(j), ready=norm_slot_1j + 2)

            out_ps = [None] * NU
            expm_ring = {}

            def emit_scores(u, i):
                c, j = UNITS[u]
                jsl = slice(j * 512, (j + 1) * 512)
                sc = scp.tile([128, 2, 512], F32, tag="sc", name=f"sc_{u}_{i}")
                for h2 in range(2):
                    hsl = slice(h2 * 64, (h2 + 1) * 64)
                    nc.tensor.matmul(
                        sc[:, h2, :],
                        kT_sb[hsl, c, i * 128 : (i + 1) * 128],
                        qT_sb[hsl, c, jsl],
                        start=True,
                        stop=True,
                    )
                expt = pexpt.tile([128, 2, 512], BF16, tag="expt", name=f"et_{u}_{i}")
                nc.scalar.activation(out=expt[:], in_=sc[:], func=EXP, scale=0.125)
                expm = pexpm.tile([128, 2, 512], BF16, tag="expm", name=f"em_{u}_{i}")
                nc.vector.tensor_tensor(
                    out=expm[:],
                    in0=expt[:],
                    in1=mask01[:, i, jsl][:, None, :].to_broadcast((128, 2, 512)),
                    op=MULT,
                )
                expm_ring[(u, i)] = expm

            def emit_attnv(u, i):
                c, j = UNITS[u]
                if i == 0:
                    out_ps[u] = opp.tile(
                        [128, 2, 512], F32, tag="ops", name=f"op_{u}"
                    )
                expm = expm_ring.pop((u, i))
                for h2 in range(2):
                    h = 2 * c + h2
                    nc.tensor.matmul(
                        out_ps[u][:, h2, :],
                        v_aug[:, i, h * 128 : (h + 1) * 128],
                        expm[:, h2, :],
                        start=(i == 0),
                        stop=(i == NSK - 1),
                    )

            def emit_norm_start(u, g):
                """Z reciprocal + gpsimd broadcast right after unit u's last
                attnV; the two DVE multiplies are spliced into the next
                slot's extras (deadline g+2: they must precede the next
                unit's first attnV, which reuses the single opp buffer)."""
                c, j = UNITS[u]
                jsl = slice(j * 512, (j + 1) * 512)
                muls = []
                for h2 in range(2):
                    zrow = pnorm.tile([1, 512], F32, tag="zrow", name=f"zw_{u}_{h2}")
                    nc.vector.tensor_copy(zrow[:], out_ps[u][64:65, h2, :])
                    zr1 = pnorm.tile([1, 512], F32, tag="zr1", name=f"z1_{u}_{h2}")
                    nc.vector.reciprocal_approx_fast(out=zr1[:], in_=zrow[:])
                    zr = pnorm.tile([64, 512], F32, tag="zr", name=f"zr_{u}_{h2}")
                    nc.gpsimd.partition_broadcast(zr[:], zr1[:])

                    def mul(u=u, c=c, h2=h2, jsl=jsl, zr=zr):
                        nc.vector.tensor_tensor(
                            out=out_cT[h2 * 64 : (h2 + 1) * 64, c, jsl],
                            in0=out_ps[u][0:64, h2, :],
                            in1=zr[:],
                            op=MULT,
                        )

                    muls.append(mul)
                add(g + 2, muls, ready=g + 1)

            norm_slot = {u * NSK + NSK - 1 + L: u for u in range(NU)}

            def pop_extras(g):
                popped = 0
                deferred = []
                while extras:
                    item = heapq.heappop(extras)
                    dl, sq, ready, true_dl, th = item
                    if ready > g:
                        deferred.append(item)
                        continue
                    assert true_dl > g, (
                        f"extras deadline miss: dl={true_dl} seq={sq} slot={g}"
                    )
                    cap = 3 if dl <= g + 2 else (2 if dl <= g + 6 else 1)
                    if popped >= cap:
                        deferred.append(item)
                        break
                    th()
                    popped += 1
                for item in deferred:
                    heapq.heappush(extras, item)

            for g in range(NSLOT + L + 1):
                u, i = divmod(g, NSK)
                if u < NU:
                    emit_scores(u, i)
                pop_extras(g)
                g2 = g - L
                if 0 <= g2 < NSLOT:
                    emit_attnv(*divmod(g2, NSK))
                if g in norm_slot:
                    emit_norm_start(norm_slot[g], g)

            # drain (deadline order keeps unit-7 norm muls before phase3(3))
            while extras:
                dl, sq, ready, true_dl, th = heapq.heappop(extras)
                th()

    nc.compile()
    return nc


def _get_nc():
    if "nc" not in _CACHE:
        _CACHE["nc"] = _build()
    return _CACHE["nc"]


def _prep_inputs(x, mask, Wq, Wk, Wv, Wo, bo):
    """Build the 8 per-core input maps (bf16 on the wire, p-major)."""
    import ml_dtypes

    bf16 = ml_dtypes.bfloat16
    x = np.asarray(x, dtype=np.float32)
    mask = np.asarray(mask, dtype=np.int32)
    wqT = np.asarray(Wq, np.float32).T
    wkT = np.asarray(Wk, np.float32).T
    wvT = np.asarray(Wv, np.float32).T
    woT = np.asarray(Wo, np.float32).T

    # x[b].T chunked: [NJ, 128, KT, 512] with xC[j, p, t, u] =
    # x[b].T[t*128+p, j*512+u]  (8KB per-partition contiguous runs)
    xCs = []
    for b in range(B):
        xT = x[b].T.astype(bf16)  # [D, S]
        xc = np.ascontiguousarray(
            xT.reshape(KT, 128, NJ, 512).transpose(2, 1, 0, 3)
        )
        xCs.append(xc)
    # keep-mask p-major: maskP[p, i, q] = (mask[b,0,q,i*128+p] == 0)
    maskPs = []
    for b in range(B):
        keepT = (mask[b, 0].T == 0).astype(bf16)  # [k, q]
        maskPs.append(
            np.ascontiguousarray(keepT.reshape(NSK, 128, S).transpose(1, 0, 2))
        )

    def wpm(wT, doff):  # [D, DPC] slice -> [128, KT, DPC]
        sl = np.ascontiguousarray(wT[:, doff : doff + DPC]).astype(bf16)
        return np.ascontiguousarray(sl.reshape(KT, 128, DPC).transpose(1, 0, 2))

    in_maps = []
    for c in range(NCORES):
        b, hq = c >> 2, c & 3
        doff = hq * DPC
        wos = np.ascontiguousarray(woT[doff : doff + DPC, :]).astype(bf16)
        in_maps.append(
            {
                "xC": xCs[b],
                "wqP": wpm(wqT, doff),
                "wkP": wpm(wkT, doff),
                "wvP": wpm(wvT, doff),
                "woP": np.ascontiguousarray(
                    wos.reshape(2, 128, D).transpose(1, 0, 2)
                ),
                "maskP": maskPs[b],
            }
        )
    return in_maps


def run(inputs: dict, trace: bool = False):
    """Run the kernel; returns (full_output, BassKernelResults)."""
    from concourse.bass_utils import run_bass_kernel_spmd

    nc = _get_nc()
    in_maps = _prep_inputs(**inputs)
    res = run_bass_kernel_spmd(
        nc, in_maps, core_ids=list(range(NCORES)), trace=trace
    )
    bo = np.asarray(inputs["bo"], dtype=np.float32)
    out = np.empty((B, S, D), dtype=np.float32)
    for b in range(B):
        acc = res.results[4 * b]["outp"].astype(np.float32)
        for hq in range(1, 4):
            acc = acc + res.results[4 * b + hq]["outp"].astype(np.float32)
        out[b] = acc + bo[None, :]
    return out, res


def kernel(**inputs) -> np.ndarray:
    out, _ = run(inputs, trace=False)
    return out
